# revision 27
# baseline (speedup 1.0000x reference)
"""BandSplit (per-band BatchNorm1d + 1x1 Conv1d) on one TRN2 chip (8 NeuronCores).

Sharding: expert-style band parallelism. Each core owns ~4 of the 31 subbands;
each band's BatchNorm (training-mode stats over (B,T)) + 1x1 conv is fully
independent, so there are no cross-core collectives.

Per core the bands are packed into two matmul "groups":
  group0: 2 big bands (K = ciA+ciB <= 50), sections of Kp=64 partitions,
          2 sections (bases 0/64); each section holds 4 batches of columns.
  group1: 1-2 small bands (K <= 32), sections of Kp=32, 4 sections
          (bases 0/32/64/96); each section holds 2 batches.
Zero-padded partition rows carry zero weights, so they contribute nothing.

On device, BatchNorm is folded into the conv:
    y = (W*diag(s)) @ x + (bias + W^T @ b2)
    s = gamma * rsqrt(var + eps),  b2 = beta - mean * s
Per-row sums come from a DVE reduce (sum) and an ACT Square-accumulate
(sum of squares); rows of different sections holding the same channel are
combined and re-broadcast by one small PE matmul against a selection matrix
that also folds in the 1/(B*T) normalization.

Matmuls are issued alternating between the two sections of a pair so each
LDWEIGHTS targets a different PE row-group than the in-flight matmul
(they overlap); PSUM is organised as four 2-bank [128, 1024] tiles, each
filled by two N=500 matmuls and drained by a single [128, 2, 500] DVE/ACT
bias-add into a bf16 staging tile.

I/O is bf16 (the 2e-2 rel-err budget is ~5x larger than bf16 quantization):
the host packs inputs into contiguous [128, 8000] bf16 shards and unpacks
bf16 outputs, so every device DMA is a full-port 2 MB transfer.
"""

import ml_dtypes
import numpy as np

SUBBANDS = [2] + [3] * 10 + [8] * 12 + [16] * 7 + [17]
BAND_START = np.concatenate([[0], np.cumsum(SUBBANDS)[:-1]]).astype(int)
C = 64
B = 8
T = 4000
EPS = 1e-5
NSUB = 500  # matmul free-dim tile

# per-core band assignment: (group0 bands, group1 bands) — indices into SUBBANDS
CORE_BANDS = [
    ([30, 11], [1, 2]),
    ([23, 12], [3, 4]),
    ([24, 13], [5, 6]),
    ([25, 14], [7, 8]),
    ([26, 15], [9, 10]),
    ([27, 16], [17, 0]),
    ([28, 18], [19, 20]),
    ([29, 21], [22]),
]

GROUP_KP = [64, 32]     # section partition size per group
GROUP_NSEC = [2, 4]     # sections per group
GROUP_NCH = [4, 2]      # [128, T] column chunks per group (2 per x tile)

# selection matrices fold the full-count normalization (each channel sees
# B*T = 32000 elements across its sections), so sel @ (sum, sqsum) = (mean, E2)
_k = np.arange(128)
SEL = [
    (((_k[:, None] % 64) == (_k[None, :] % 64)).astype(np.float32) / 32000.0),
    (((_k[:, None] % 32) == (_k[None, :] % 32)).astype(np.float32) / 32000.0),
]

_CACHE = {}


def _build_nc():
    from concourse import bacc, mybir
    import concourse.tile as tile

    f32 = mybir.dt.float32
    bf16 = mybir.dt.bfloat16
    nc = bacc.Bacc("TRN2", target_bir_lowering=False, debug=False, num_devices=8)

    xg = [
        nc.dram_tensor("xg0", [4, 128, T], bf16, kind="ExternalInput"),
        nc.dram_tensor("xg1", [2, 128, T], bf16, kind="ExternalInput"),
    ]
    w_d = nc.dram_tensor("w", [128, 256], bf16, kind="ExternalInput")
    sel_d = nc.dram_tensor("sel", [128, 256], f32, kind="ExternalInput")
    gbb_d = nc.dram_tensor("gbb", [128, 6], f32, kind="ExternalInput")
    id_d = nc.dram_tensor("ident", [128, 128], bf16, kind="ExternalInput")
    y_d = [
        nc.dram_tensor("y0", [4, 1, 128, 2 * T], bf16, kind="ExternalOutput"),
        nc.dram_tensor("y1", [2, 2, 128, 2 * T], bf16, kind="ExternalOutput"),
    ]

    with tile.TileContext(nc) as tc, \
         tc.tile_pool(name="xpool", bufs=1) as xpool, \
         tc.tile_pool(name="consts", bufs=1) as consts, \
         tc.tile_pool(name="statsp", bufs=1) as statsp, \
         tc.tile_pool(name="vecs", bufs=1) as vecs, \
         tc.tile_pool(name="wfp", bufs=1) as wfp, \
         tc.tile_pool(name="ostage", bufs=5) as ostage, \
         tc.tile_pool(name="psmm", bufs=4, space="PSUM") as psmm:

        alt = [0]

        xtiles = {}
        wfs = {}
        bfs = {}
        w_t = consts.tile([128, 256], bf16, tag="w")
        sel_t = consts.tile([128, 256], f32, tag="sel")
        gbb_t = consts.tile([128, 6], f32, tag="gbb")
        id_t = consts.tile([128, 128], bf16, tag="ident")
        eps_t = consts.tile([128, 1], f32, tag="eps")

        def emit_dmas(g):
            kp, nsec, nch = GROUP_KP[g], GROUP_NSEC[g], GROUP_NCH[g]
            xts = []
            for i in range(nch):
                xt = xpool.tile([128, T], bf16, tag=f"x{g}_{i}",
                                name=f"xt{g}_{i}")
                eng = nc.scalar if i % 2 == 0 else nc.sync
                eng.dma_start(out=xt[:], in_=xg[g][i])
                xts.append(xt)
            xtiles[g] = xts

        def emit_consts():
            nc.sync.dma_start(out=id_t[:], in_=id_d[:])
            nc.sync.dma_start(out=w_t[:], in_=w_d[:])
            nc.sync.dma_start(out=sel_t[:], in_=sel_d[:])
            nc.sync.dma_start(out=gbb_t[:], in_=gbb_d[:])
            nc.vector.memset(eps_t[:], EPS)

        sums_t = {}
        sv_t = {}

        def emit_stats_mm(g):
            kp, nsec, nch = GROUP_KP[g], GROUP_NSEC[g], GROUP_NCH[g]
            sums = statsp.tile([128, 2, nch * 2], f32, tag=f"sums{g}",
                               name=f"sums{g}")
            sums_t[g] = sums
            for c in range(nch):
                pssum = psmm.tile([128, 512], f32, tag="mm",
                                  name=f"pssum{g}_{c}")
                for j in range(8):
                    nc.tensor.matmul(
                        pssum[:, 0:NSUB], id_t[:],
                        xtiles[g][c][:, j * NSUB:(j + 1) * NSUB],
                        start=(j == 0), stop=(j == 7),
                        tile_position=(0, 0))
                nc.vector.tensor_reduce(
                    out=sums[:, 0, c:c + 1], in_=pssum[:, 0:NSUB],
                    op=mybir.AluOpType.add, axis=mybir.AxisListType.X)

        def emit_stats_sq(g, c, split):
            sums = sums_t[g]
            for p2 in range(2):
                scr_v = statsp.tile([128, 2000], bf16, tag="scr_v", bufs=3,
                                    name=f"scrv{g}_{c}_{p2}")
                xin = xtiles[g][c][:, p2 * 2000:(p2 + 1) * 2000]
                acc = sums[:, 1, 2 * c + p2:2 * c + p2 + 1]
                if split and p2 == 1:
                    nc.vector.scalar_tensor_tensor(
                        out=scr_v[:], in0=xin, scalar=0.0, in1=xin,
                        op0=mybir.AluOpType.add, op1=mybir.AluOpType.mult,
                        accum_out=acc)
                else:
                    nc.scalar.activation(
                        out=scr_v[:], in_=xin,
                        func=mybir.ActivationFunctionType.Square,
                        bias=0.0, scale=1.0, accum_out=acc)

        def emit_sv_fold(g):
            kp, nsec, nch = GROUP_KP[g], GROUP_NSEC[g], GROUP_NCH[g]
            wg = w_t[:, 128 * g:128 * (g + 1)]
            selg = sel_t[:, 128 * g:128 * (g + 1)]
            biag = gbb_t[:, 3 * g + 2:3 * g + 3]
            sums = sums_t[g]
            sv = vecs.tile([128, 2], f32, tag=f"sv{g}", name=f"sv{g}")
            for j, width in ((0, nch), (1, nch * 2)):
                scr_sv = vecs.tile([128, nch * 2], f32, tag=f"scrsv{g}_{j}",
                                   name=f"scrsv{g}_{j}")
                nc.scalar.activation(
                    out=scr_sv[:, 0:width], in_=sums[:, j, 0:width],
                    func=mybir.ActivationFunctionType.Identity,
                    bias=0.0, scale=1.0, accum_out=sv[:, j:j + 1])
            # combine across sections + broadcast back via selection matmul;
            # result is (mean, E[x^2]) per partition row
            pst = psmm.tile([128, 2], f32, tag="mm", name=f"pst{g}")
            nc.tensor.matmul(pst[:], selg, sv[:], start=True, stop=True)

            # fold BN into conv (gamma/beta are folded on the host: w is
            # W*gamma and bias2 = bias + W^T beta)
            msq2 = vecs.tile([128, 1], f32, tag=f"msq2{g}", name=f"msq2{g}")
            nc.scalar.activation(out=msq2[:], in_=pst[:, 0:1],
                                 func=mybir.ActivationFunctionType.Square,
                                 bias=0.0, scale=1.0)
            var = vecs.tile([128, 1], f32, tag=f"var{g}", name=f"var{g}")
            nc.vector.tensor_sub(out=var[:], in0=pst[:, 1:2], in1=msq2[:])
            std = vecs.tile([128, 1], f32, tag=f"std{g}", name=f"std{g}")
            nc.scalar.activation(out=std[:], in_=var[:],
                                 func=mybir.ActivationFunctionType.Sqrt,
                                 bias=eps_t[:], scale=1.0)
            rstd = vecs.tile([128, 1], f32, tag=f"rstd{g}", name=f"rstd{g}")
            nc.vector.reciprocal(out=rstd[:], in_=std[:])
            wf = wfp.tile([128, 128], bf16, tag=f"wf{g}", name=f"wf{g}")
            nc.vector.tensor_scalar_mul(out=wf[:], in0=wg, scalar1=rstd[:])
            mr = vecs.tile([128, 1], bf16, tag=f"mr{g}", name=f"mr{g}")
            nc.vector.tensor_copy(out=mr[:], in_=pst[:, 0:1])
            psb = psmm.tile([128, 1], f32, tag="mm", name=f"psb{g}")
            nc.tensor.matmul(psb[:], wf[0:kp, :], mr[0:kp, :],
                             start=True, stop=True)
            bf = vecs.tile([128, 1], f32, tag=f"bf{g}", name=f"bf{g}")
            nc.vector.tensor_sub(out=bf[:], in0=biag, in1=psb[:])
            wfs[g] = wf
            bfs[g] = bf

        def emit_main(g, c, qp):
            # matmuls alternate between the two sections of a pair so each
            # LDWEIGHTS hits a different PE row-group than the running matmul
            kp, nsec, nch = GROUP_KP[g], GROUP_NSEC[g], GROUP_NCH[g]
            wf, bf = wfs[g], bfs[g]
            xts = xtiles[g]
            qs = (2 * qp, 2 * qp + 1)
            stage = ostage.tile([128, 2 * T], bf16, tag="stage",
                                name=f"stage{g}_{c}_{qp}")
            for u2 in range(4):
                pss = [psmm.tile([128, 1024], f32, tag="mm",
                                 name=f"ps{g}_{c}_{qp}_{u2}_{qi}")
                       for qi in range(2)]
                for h in range(2):
                    u = u2 * 2 + h
                    for qi, q in enumerate(qs):
                        base = kp * q
                        nc.tensor.matmul(
                            pss[qi][:, 512 * h:512 * h + NSUB],
                            wf[base:base + kp, :],
                            xts[c][base:base + kp,
                                   u * NSUB:(u + 1) * NSUB],
                            start=True, stop=True,
                            tile_position=(base, 0),
                        )
                for qi in range(2):
                    pv = pss[qi][:].rearrange(
                        "p (a b) -> p a b", a=2)[:, :, 0:NSUB]
                    so = stage[:, qi * T + u2 * 1000:
                               qi * T + (u2 + 1) * 1000].rearrange(
                        "p (a b) -> p a b", a=2)
                    if alt[0] % 8 in (0, 2, 4, 6):
                        nc.vector.tensor_scalar_add(out=so, in0=pv,
                                                    scalar1=bf[:])
                    else:
                        nc.scalar.add(out=so, in_=pv, add=bf[:])
                    alt[0] += 1
                # drain the finished 1000-col block (both q halves) early
                lo = u2 * 1000
                svw = stage[:].rearrange(
                    "p (q n) -> p q n", q=2)[:, :, lo:lo + 1000]
                dvw = y_d[g][c, qp].rearrange(
                    "p (q n) -> p q n", q=2)[:, :, lo:lo + 1000]
                eng = nc.gpsimd if u2 % 2 else nc.sync
                eng.dma_start(out=dvw, in_=svw)

        emit_dmas(1)
        emit_consts()
        emit_dmas(0)
        emit_stats_mm(1)
        for c in range(2):
            emit_stats_sq(1, c, split=False)
        emit_sv_fold(1)
        emit_stats_mm(0)
        g1_blocks = [(c, qp) for c in range(GROUP_NCH[1])
                     for qp in range(GROUP_NSEC[1] // 2)]
        for i, (c, qp) in enumerate(g1_blocks):
            emit_main(1, c, qp)
            if i < GROUP_NCH[0]:
                emit_stats_sq(0, i, split=True)
        emit_sv_fold(0)
        for c in range(GROUP_NCH[0]):
            for qp in range(GROUP_NSEC[0] // 2):
                emit_main(0, c, qp)

    nc.compile()
    return nc


def _band_x(spec, i):
    s, sb = BAND_START[i], SUBBANDS[i]
    return spec[:, s:s + sb].reshape(B, 2 * sb, T)


def _make_in_maps(spec, weights, biases, gammas, betas):
    in_maps = []
    for core in range(8):
        im = {}
        w_all = np.zeros((128, 256), np.float32)
        gbb = np.zeros((128, 6), np.float32)
        for g, bands in enumerate(CORE_BANDS[core]):
            kp, nsec, nch = GROUP_KP[g], GROUP_NSEC[g], GROUP_NCH[g]
            xcat = np.concatenate([_band_x(spec, i) for i in bands], axis=1)
            K = xcat.shape[1]
            xgh = np.zeros((nch, 128, T), np.float32)
            for q in range(nsec):
                # section q (partitions kp*q..kp*q+K), chunk c -> batch nch*q+c
                xgh[:, kp * q:kp * q + K, :] = xcat[nch * q:nch * (q + 1)]
            im[f"xg{g}"] = xgh.astype(ml_dtypes.bfloat16)

            blk = np.zeros((kp, 128), np.float32)
            biasv = np.zeros((128,), np.float32)
            off = 0
            for bi, band in enumerate(bands):
                ci = 2 * SUBBANDS[band]
                wb = np.asarray(weights[band], np.float64)
                gb = np.asarray(gammas[band], np.float64)
                bb = np.asarray(betas[band], np.float64)
                blk[off:off + ci, 64 * bi:64 * bi + 64] = (wb * gb).T
                biasv[64 * bi:64 * bi + 64] = (
                    np.asarray(biases[band], np.float64) + wb @ bb
                ).astype(np.float32)
                off += ci
            for q in range(128 // kp):
                w_all[kp * q:kp * (q + 1), 128 * g:128 * (g + 1)] = blk
            gbb[:, 3 * g + 2] = biasv
        im["w"] = w_all.astype(ml_dtypes.bfloat16)
        im["ident"] = np.eye(128, dtype=np.float32).astype(ml_dtypes.bfloat16)
        im["sel"] = np.concatenate(SEL, axis=1)
        im["gbb"] = gbb
        in_maps.append(im)
    return in_maps


def _gather(results):
    out = np.empty((B, 31, C, T), np.float32)
    for core in range(8):
        for g, bands in enumerate(CORE_BANDS[core]):
            nch = GROUP_NCH[g]
            # y_g: [nch, nsec//2, 128, 2, T]; col half j -> batch nch*(2qp+j)+c
            yg = np.asarray(results[core][f"y{g}"]).astype(np.float32)
            yg = yg.reshape(nch, -1, 128, 2, T)
            for bi, band in enumerate(bands):
                rows = yg[:, :, 64 * bi:64 * bi + 64]  # [nch, nsec2, 64, 2, T]
                for c in range(nch):
                    for qp in range(rows.shape[1]):
                        for j in range(2):
                            out[nch * (2 * qp + j) + c, band] = rows[c, qp, :, j]
    return out


def run(trace=False, trace_cores=None, **inputs):
    from concourse.bass_utils import run_bass_kernel_spmd

    spec = np.ascontiguousarray(np.asarray(inputs["spec_noisy"], np.float32))
    in_maps = _make_in_maps(spec, inputs["weights"], inputs["biases"],
                            inputs["gammas"], inputs["betas"])
    if "nc" not in _CACHE:
        _CACHE["nc"] = _build_nc()
    nc = _CACHE["nc"]
    res = run_bass_kernel_spmd(nc, in_maps, core_ids=list(range(8)),
                               trace=trace, trace_cores=trace_cores)
    return _gather(res.results), res


def kernel(**inputs):
    out, _ = run(trace=False, **inputs)
    return out


# revision 28
# speedup vs baseline: 1.0144x; 1.0144x over previous
"""BandSplit (per-band BatchNorm1d + 1x1 Conv1d) on one TRN2 chip (8 NeuronCores).

Sharding: expert-style band parallelism. Each core owns ~4 of the 31 subbands;
each band's BatchNorm (training-mode stats over (B,T)) + 1x1 conv is fully
independent, so there are no cross-core collectives.

Per core the bands are packed into two matmul "groups":
  group0: 2 big bands (K = ciA+ciB <= 50), sections of Kp=64 partitions,
          2 sections (bases 0/64); each section holds 4 batches of columns.
  group1: 1-2 small bands (K <= 32), sections of Kp=32, 4 sections
          (bases 0/32/64/96); each section holds 2 batches.
Zero-padded partition rows carry zero weights, so they contribute nothing.

On device, BatchNorm is folded into the conv:
    y = (W*diag(s)) @ x + (bias + W^T @ b2)
    s = gamma * rsqrt(var + eps),  b2 = beta - mean * s
Per-row sums come from a DVE reduce (sum) and an ACT Square-accumulate
(sum of squares); rows of different sections holding the same channel are
combined and re-broadcast by one small PE matmul against a selection matrix
that also folds in the 1/(B*T) normalization.

Matmuls are issued alternating between the two sections of a pair so each
LDWEIGHTS targets a different PE row-group than the in-flight matmul
(they overlap); PSUM is organised as four 2-bank [128, 1024] tiles, each
filled by two N=500 matmuls and drained by a single [128, 2, 500] DVE/ACT
bias-add into a bf16 staging tile.

I/O is bf16 (the 2e-2 rel-err budget is ~5x larger than bf16 quantization):
the host packs inputs into contiguous [128, 8000] bf16 shards and unpacks
bf16 outputs, so every device DMA is a full-port 2 MB transfer.
"""

import ml_dtypes
import numpy as np

SUBBANDS = [2] + [3] * 10 + [8] * 12 + [16] * 7 + [17]
BAND_START = np.concatenate([[0], np.cumsum(SUBBANDS)[:-1]]).astype(int)
C = 64
B = 8
T = 4000
EPS = 1e-5
NSUB = 500  # matmul free-dim tile

# per-core band assignment: (group0 bands, group1 bands) — indices into SUBBANDS
CORE_BANDS = [
    ([30, 11], [1, 2]),
    ([23, 12], [3, 4]),
    ([24, 13], [5, 6]),
    ([25, 14], [7, 8]),
    ([26, 15], [9, 10]),
    ([27, 16], [17, 0]),
    ([28, 18], [19, 20]),
    ([29, 21], [22]),
]

GROUP_KP = [64, 32]     # section partition size per group
GROUP_NSEC = [2, 4]     # sections per group
GROUP_NCH = [4, 2]      # [128, T] column chunks per group (2 per x tile)

# selection matrices fold the full-count normalization (each channel sees
# B*T = 32000 elements across its sections), so sel @ (sum, sqsum) = (mean, E2)
_k = np.arange(128)
SEL = [
    (((_k[:, None] % 64) == (_k[None, :] % 64)).astype(np.float32) / 32000.0),
    (((_k[:, None] % 32) == (_k[None, :] % 32)).astype(np.float32) / 32000.0),
]

_CACHE = {}


def _build_nc():
    from concourse import bacc, mybir
    import concourse.tile as tile

    f32 = mybir.dt.float32
    bf16 = mybir.dt.bfloat16
    nc = bacc.Bacc("TRN2", target_bir_lowering=False, debug=False, num_devices=8)

    xg = [
        nc.dram_tensor("xg0", [4, 128, T], bf16, kind="ExternalInput"),
        nc.dram_tensor("xg1", [2, 128, T], bf16, kind="ExternalInput"),
    ]
    w_d = nc.dram_tensor("w", [128, 256], bf16, kind="ExternalInput")
    sel_d = nc.dram_tensor("sel", [128, 256], f32, kind="ExternalInput")
    gbb_d = nc.dram_tensor("gbb", [128, 6], f32, kind="ExternalInput")
    id_d = nc.dram_tensor("ident", [128, 128], bf16, kind="ExternalInput")
    y_d = [
        nc.dram_tensor("y0", [4, 1, 128, 2 * T], bf16, kind="ExternalOutput"),
        nc.dram_tensor("y1", [2, 2, 128, 2 * T], bf16, kind="ExternalOutput"),
    ]

    with tile.TileContext(nc) as tc, \
         tc.tile_pool(name="xpool", bufs=1) as xpool, \
         tc.tile_pool(name="consts", bufs=1) as consts, \
         tc.tile_pool(name="statsp", bufs=1) as statsp, \
         tc.tile_pool(name="vecs", bufs=1) as vecs, \
         tc.tile_pool(name="wfp", bufs=1) as wfp, \
         tc.tile_pool(name="ostage", bufs=5) as ostage, \
         tc.tile_pool(name="psmm", bufs=4, space="PSUM") as psmm:

        alt = [0]

        xtiles = {}
        wfs = {}
        bfs = {}
        w_t = consts.tile([128, 256], bf16, tag="w")
        sel_t = consts.tile([128, 256], f32, tag="sel")
        gbb_t = consts.tile([128, 6], f32, tag="gbb")
        id_t = consts.tile([128, 128], bf16, tag="ident")
        eps_t = consts.tile([128, 1], f32, tag="eps")

        def emit_dmas(g):
            kp, nsec, nch = GROUP_KP[g], GROUP_NSEC[g], GROUP_NCH[g]
            xts = []
            for i in range(nch):
                xt = xpool.tile([128, T], bf16, tag=f"x{g}_{i}",
                                name=f"xt{g}_{i}")
                eng = nc.scalar if i % 2 == 0 else nc.sync
                eng.dma_start(out=xt[:], in_=xg[g][i])
                xts.append(xt)
            xtiles[g] = xts

        def emit_consts():
            nc.sync.dma_start(out=id_t[:], in_=id_d[:])
            nc.sync.dma_start(out=w_t[:], in_=w_d[:])
            nc.sync.dma_start(out=sel_t[:], in_=sel_d[:])
            nc.sync.dma_start(out=gbb_t[:], in_=gbb_d[:])
            nc.vector.memset(eps_t[:], EPS)

        sums_t = {}
        sv_t = {}

        def emit_stats_mm(g):
            kp, nsec, nch = GROUP_KP[g], GROUP_NSEC[g], GROUP_NCH[g]
            sums = statsp.tile([128, 2, nch * 2], f32, tag=f"sums{g}",
                               name=f"sums{g}")
            sums_t[g] = sums
            for c in range(nch):
                pssum = psmm.tile([128, 512], f32, tag="mm",
                                  name=f"pssum{g}_{c}")
                for j in range(8):
                    nc.tensor.matmul(
                        pssum[:, 0:NSUB], id_t[:],
                        xtiles[g][c][:, j * NSUB:(j + 1) * NSUB],
                        start=(j == 0), stop=(j == 7),
                        tile_position=(0, 0))
                nc.vector.tensor_reduce(
                    out=sums[:, 0, c:c + 1], in_=pssum[:, 0:NSUB],
                    op=mybir.AluOpType.add, axis=mybir.AxisListType.X)

        def emit_stats_sq(g, c, split):
            sums = sums_t[g]
            for p2 in range(2):
                scr_v = statsp.tile([128, 2000], bf16, tag="scr_v", bufs=3,
                                    name=f"scrv{g}_{c}_{p2}")
                xin = xtiles[g][c][:, p2 * 2000:(p2 + 1) * 2000]
                acc = sums[:, 1, 2 * c + p2:2 * c + p2 + 1]
                if split and p2 == 1:
                    nc.vector.scalar_tensor_tensor(
                        out=scr_v[:], in0=xin, scalar=0.0, in1=xin,
                        op0=mybir.AluOpType.add, op1=mybir.AluOpType.mult,
                        accum_out=acc)
                else:
                    nc.scalar.activation(
                        out=scr_v[:], in_=xin,
                        func=mybir.ActivationFunctionType.Square,
                        bias=0.0, scale=1.0, accum_out=acc)

        def emit_sv_fold(g):
            kp, nsec, nch = GROUP_KP[g], GROUP_NSEC[g], GROUP_NCH[g]
            wg = w_t[:, 128 * g:128 * (g + 1)]
            selg = sel_t[:, 128 * g:128 * (g + 1)]
            biag = gbb_t[:, 3 * g + 2:3 * g + 3]
            sums = sums_t[g]
            sv = vecs.tile([128, 2], f32, tag=f"sv{g}", name=f"sv{g}")
            for j, width in ((0, nch), (1, nch * 2)):
                scr_sv = vecs.tile([128, nch * 2], f32, tag=f"scrsv{g}_{j}",
                                   name=f"scrsv{g}_{j}")
                nc.scalar.activation(
                    out=scr_sv[:, 0:width], in_=sums[:, j, 0:width],
                    func=mybir.ActivationFunctionType.Identity,
                    bias=0.0, scale=1.0, accum_out=sv[:, j:j + 1])
            # combine across sections + broadcast back via selection matmul;
            # result is (mean, E[x^2]) per partition row
            pst = psmm.tile([128, 2], f32, tag="mm", name=f"pst{g}")
            nc.tensor.matmul(pst[:], selg, sv[:], start=True, stop=True)

            # fold BN into conv (gamma/beta are folded on the host: w is
            # W*gamma and bias2 = bias + W^T beta)
            msq2 = vecs.tile([128, 1], f32, tag=f"msq2{g}", name=f"msq2{g}")
            nc.scalar.activation(out=msq2[:], in_=pst[:, 0:1],
                                 func=mybir.ActivationFunctionType.Square,
                                 bias=0.0, scale=1.0)
            var = vecs.tile([128, 1], f32, tag=f"var{g}", name=f"var{g}")
            nc.vector.tensor_sub(out=var[:], in0=pst[:, 1:2], in1=msq2[:])
            std = vecs.tile([128, 1], f32, tag=f"std{g}", name=f"std{g}")
            nc.scalar.activation(out=std[:], in_=var[:],
                                 func=mybir.ActivationFunctionType.Sqrt,
                                 bias=eps_t[:], scale=1.0)
            rstd = vecs.tile([128, 1], f32, tag=f"rstd{g}", name=f"rstd{g}")
            nc.vector.reciprocal(out=rstd[:], in_=std[:])
            wf = wfp.tile([128, 128], bf16, tag=f"wf{g}", name=f"wf{g}")
            nc.vector.tensor_scalar_mul(out=wf[:], in0=wg, scalar1=rstd[:])
            mr = vecs.tile([128, 1], bf16, tag=f"mr{g}", name=f"mr{g}")
            nc.vector.tensor_copy(out=mr[:], in_=pst[:, 0:1])
            psb = psmm.tile([128, 1], f32, tag="mm", name=f"psb{g}")
            nc.tensor.matmul(psb[:], wf[0:kp, :], mr[0:kp, :],
                             start=True, stop=True)
            bf = vecs.tile([128, 1], f32, tag=f"bf{g}", name=f"bf{g}")
            nc.vector.tensor_sub(out=bf[:], in0=biag, in1=psb[:])
            wfs[g] = wf
            bfs[g] = bf

        def emit_main(g, c, qp):
            # matmuls alternate between the two sections of a pair so each
            # LDWEIGHTS hits a different PE row-group than the running matmul
            kp, nsec, nch = GROUP_KP[g], GROUP_NSEC[g], GROUP_NCH[g]
            wf, bf = wfs[g], bfs[g]
            xts = xtiles[g]
            qs = (2 * qp, 2 * qp + 1)
            stage = ostage.tile([128, 2 * T], bf16, tag="stage",
                                name=f"stage{g}_{c}_{qp}")
            for u2 in range(4):
                pss = [psmm.tile([128, 1024], f32, tag="mm",
                                 name=f"ps{g}_{c}_{qp}_{u2}_{qi}")
                       for qi in range(2)]
                for h in range(2):
                    u = u2 * 2 + h
                    for qi, q in enumerate(qs):
                        base = kp * q
                        nc.tensor.matmul(
                            pss[qi][:, 512 * h:512 * h + NSUB],
                            wf[base:base + kp, :],
                            xts[c][base:base + kp,
                                   u * NSUB:(u + 1) * NSUB],
                            start=True, stop=True,
                            tile_position=(base, 0),
                        )
                for qi in range(2):
                    pv = pss[qi][:].rearrange(
                        "p (a b) -> p a b", a=2)[:, :, 0:NSUB]
                    so = stage[:, qi * T + u2 * 1000:
                               qi * T + (u2 + 1) * 1000].rearrange(
                        "p (a b) -> p a b", a=2)
                    if alt[0] % 8 in (0, 2, 4, 6):
                        nc.vector.tensor_scalar_add(out=so, in0=pv,
                                                    scalar1=bf[:])
                    else:
                        nc.scalar.add(out=so, in_=pv, add=bf[:])
                    alt[0] += 1
                if u2 == 1 or u2 == 3:
                    # drain the finished u2-pair (both q halves) early
                    lo = (u2 - 1) * 1000
                    svw = stage[:].rearrange(
                        "p (q n) -> p q n", q=2)[:, :, lo:lo + 2000]
                    dvw = y_d[g][c, qp].rearrange(
                        "p (q n) -> p q n", q=2)[:, :, lo:lo + 2000]
                    nc.gpsimd.dma_start(out=dvw, in_=svw)

        emit_dmas(1)
        emit_consts()
        emit_dmas(0)
        emit_stats_mm(1)
        for c in range(2):
            emit_stats_sq(1, c, split=False)
        emit_sv_fold(1)
        emit_stats_mm(0)
        g1_blocks = [(c, qp) for c in range(GROUP_NCH[1])
                     for qp in range(GROUP_NSEC[1] // 2)]
        for i, (c, qp) in enumerate(g1_blocks):
            emit_main(1, c, qp)
            if i < GROUP_NCH[0]:
                emit_stats_sq(0, i, split=True)
        emit_sv_fold(0)
        for c in range(GROUP_NCH[0]):
            for qp in range(GROUP_NSEC[0] // 2):
                emit_main(0, c, qp)

    nc.compile()
    return nc


def _band_x(spec, i):
    s, sb = BAND_START[i], SUBBANDS[i]
    return spec[:, s:s + sb].reshape(B, 2 * sb, T)


def _make_in_maps(spec, weights, biases, gammas, betas):
    in_maps = []
    for core in range(8):
        im = {}
        w_all = np.zeros((128, 256), np.float32)
        gbb = np.zeros((128, 6), np.float32)
        for g, bands in enumerate(CORE_BANDS[core]):
            kp, nsec, nch = GROUP_KP[g], GROUP_NSEC[g], GROUP_NCH[g]
            xcat = np.concatenate([_band_x(spec, i) for i in bands], axis=1)
            K = xcat.shape[1]
            xgh = np.zeros((nch, 128, T), np.float32)
            for q in range(nsec):
                # section q (partitions kp*q..kp*q+K), chunk c -> batch nch*q+c
                xgh[:, kp * q:kp * q + K, :] = xcat[nch * q:nch * (q + 1)]
            im[f"xg{g}"] = xgh.astype(ml_dtypes.bfloat16)

            blk = np.zeros((kp, 128), np.float32)
            biasv = np.zeros((128,), np.float32)
            off = 0
            for bi, band in enumerate(bands):
                ci = 2 * SUBBANDS[band]
                wb = np.asarray(weights[band], np.float64)
                gb = np.asarray(gammas[band], np.float64)
                bb = np.asarray(betas[band], np.float64)
                blk[off:off + ci, 64 * bi:64 * bi + 64] = (wb * gb).T
                biasv[64 * bi:64 * bi + 64] = (
                    np.asarray(biases[band], np.float64) + wb @ bb
                ).astype(np.float32)
                off += ci
            for q in range(128 // kp):
                w_all[kp * q:kp * (q + 1), 128 * g:128 * (g + 1)] = blk
            gbb[:, 3 * g + 2] = biasv
        im["w"] = w_all.astype(ml_dtypes.bfloat16)
        im["ident"] = np.eye(128, dtype=np.float32).astype(ml_dtypes.bfloat16)
        im["sel"] = np.concatenate(SEL, axis=1)
        im["gbb"] = gbb
        in_maps.append(im)
    return in_maps


def _gather(results):
    out = np.empty((B, 31, C, T), np.float32)
    for core in range(8):
        for g, bands in enumerate(CORE_BANDS[core]):
            nch = GROUP_NCH[g]
            # y_g: [nch, nsec//2, 128, 2, T]; col half j -> batch nch*(2qp+j)+c
            yg = np.asarray(results[core][f"y{g}"]).astype(np.float32)
            yg = yg.reshape(nch, -1, 128, 2, T)
            for bi, band in enumerate(bands):
                rows = yg[:, :, 64 * bi:64 * bi + 64]  # [nch, nsec2, 64, 2, T]
                for c in range(nch):
                    for qp in range(rows.shape[1]):
                        for j in range(2):
                            out[nch * (2 * qp + j) + c, band] = rows[c, qp, :, j]
    return out


def run(trace=False, trace_cores=None, **inputs):
    from concourse.bass_utils import run_bass_kernel_spmd

    spec = np.ascontiguousarray(np.asarray(inputs["spec_noisy"], np.float32))
    in_maps = _make_in_maps(spec, inputs["weights"], inputs["biases"],
                            inputs["gammas"], inputs["betas"])
    if "nc" not in _CACHE:
        _CACHE["nc"] = _build_nc()
    nc = _CACHE["nc"]
    res = run_bass_kernel_spmd(nc, in_maps, core_ids=list(range(8)),
                               trace=trace, trace_cores=trace_cores)
    return _gather(res.results), res


def kernel(**inputs):
    out, _ = run(trace=False, **inputs)
    return out


# revision 29
# speedup vs baseline: 1.1341x; 1.1180x over previous
"""BandSplit (per-band BatchNorm1d + 1x1 Conv1d) on one TRN2 chip (8 NeuronCores).

Sharding: expert-style band parallelism. Each core owns ~4 of the 31 subbands;
each band's BatchNorm (training-mode stats over (B,T)) + 1x1 conv is fully
independent, so there are no cross-core collectives.

Per core the bands are packed into two matmul "groups":
  group0: 2 big bands (K = ciA+ciB <= 50), sections of Kp=64 partitions,
          2 sections (bases 0/64); each section holds 4 batches of columns.
  group1: 1-2 small bands (K <= 32), sections of Kp=32, 4 sections
          (bases 0/32/64/96); each section holds 2 batches.
Zero-padded partition rows carry zero weights, so they contribute nothing.

On device, BatchNorm is folded into the conv:
    y = (W*diag(s)) @ x + (bias + W^T @ b2)
    s = gamma * rsqrt(var + eps),  b2 = beta - mean * s
Per-row sums come from a DVE reduce (sum) and an ACT Square-accumulate
(sum of squares); rows of different sections holding the same channel are
combined and re-broadcast by one small PE matmul against a selection matrix
that also folds in the 1/(B*T) normalization.

Matmuls are issued alternating between the two sections of a pair so each
LDWEIGHTS targets a different PE row-group than the in-flight matmul
(they overlap); PSUM is organised as four 2-bank [128, 1024] tiles, each
filled by two N=500 matmuls and drained by a single [128, 2, 500] DVE/ACT
bias-add into a bf16 staging tile.

I/O is bf16 (the 2e-2 rel-err budget is ~5x larger than bf16 quantization):
the host packs inputs into contiguous [128, 8000] bf16 shards and unpacks
bf16 outputs, so every device DMA is a full-port 2 MB transfer.
"""

import ml_dtypes
import numpy as np

SUBBANDS = [2] + [3] * 10 + [8] * 12 + [16] * 7 + [17]
BAND_START = np.concatenate([[0], np.cumsum(SUBBANDS)[:-1]]).astype(int)
C = 64
B = 8
T = 4000
EPS = 1e-5
NSUB = 500  # matmul free-dim tile

# per-core band assignment: (group0 bands, group1 bands) — indices into SUBBANDS
CORE_BANDS = [
    ([30, 11], [1, 2]),
    ([23, 12], [3, 4]),
    ([24, 13], [5, 6]),
    ([25, 14], [7, 8]),
    ([26, 15], [9, 10]),
    ([27, 16], [17, 0]),
    ([28, 18], [19, 20]),
    ([29, 21], [22]),
]

GROUP_KP = [64, 32]     # section partition size per group
GROUP_NSEC = [2, 4]     # sections per group
GROUP_NCH = [4, 2]      # [128, T] column chunks per group (2 per x tile)

# selection matrices fold the full-count normalization (each channel sees
# B*T = 32000 elements across its sections), so sel @ (sum, sqsum) = (mean, E2)
_k = np.arange(128)
SEL = [
    (((_k[:, None] % 64) == (_k[None, :] % 64)).astype(np.float32) / 32000.0),
    (((_k[:, None] % 32) == (_k[None, :] % 32)).astype(np.float32) / 32000.0),
]

_CACHE = {}


def _build_nc():
    from concourse import bacc, mybir
    import concourse.tile as tile

    f32 = mybir.dt.float32
    bf16 = mybir.dt.bfloat16
    nc = bacc.Bacc("TRN2", target_bir_lowering=False, debug=False, num_devices=8)

    xg = [
        nc.dram_tensor("xg0", [4, 128, T], bf16, kind="ExternalInput"),
        nc.dram_tensor("xg1", [2, 128, T], bf16, kind="ExternalInput"),
    ]
    w_d = nc.dram_tensor("w", [128, 256], bf16, kind="ExternalInput")
    sel_d = nc.dram_tensor("sel", [128, 256], f32, kind="ExternalInput")
    gbb_d = nc.dram_tensor("gbb", [128, 6], f32, kind="ExternalInput")
    id_d = nc.dram_tensor("ident", [128, 128], bf16, kind="ExternalInput")
    y_d = [
        nc.dram_tensor("y0", [4, 1, 128, 2 * T], bf16, kind="ExternalOutput"),
        nc.dram_tensor("y1", [2, 2, 128, 2 * T], bf16, kind="ExternalOutput"),
    ]

    with tile.TileContext(nc) as tc, \
         tc.tile_pool(name="xpool", bufs=1) as xpool, \
         tc.tile_pool(name="consts", bufs=1) as consts, \
         tc.tile_pool(name="statsp", bufs=1) as statsp, \
         tc.tile_pool(name="vecs", bufs=1) as vecs, \
         tc.tile_pool(name="wfp", bufs=1) as wfp, \
         tc.tile_pool(name="ostage", bufs=5) as ostage, \
         tc.tile_pool(name="psmm", bufs=4, space="PSUM") as psmm:

        alt = [0]

        xtiles = {}
        wfs = {}
        bfs = {}
        w_t = consts.tile([128, 256], bf16, tag="w")
        sel_t = consts.tile([128, 256], f32, tag="sel")
        gbb_t = consts.tile([128, 6], f32, tag="gbb")
        id_t = consts.tile([128, 128], bf16, tag="ident")
        eps_t = consts.tile([128, 1], f32, tag="eps")

        def emit_dmas(g):
            kp, nsec, nch = GROUP_KP[g], GROUP_NSEC[g], GROUP_NCH[g]
            xts = []
            for i in range(nch):
                xt = xpool.tile([128, T], bf16, tag=f"x{g}_{i}",
                                name=f"xt{g}_{i}")
                eng = nc.scalar if i % 2 == 0 else nc.sync
                eng.dma_start(out=xt[:], in_=xg[g][i])
                xts.append(xt)
            xtiles[g] = xts

        def emit_consts():
            nc.sync.dma_start(out=id_t[:], in_=id_d[:])
            nc.sync.dma_start(out=w_t[:], in_=w_d[:])
            nc.sync.dma_start(out=sel_t[:], in_=sel_d[:])
            nc.sync.dma_start(out=gbb_t[:], in_=gbb_d[:])
            nc.vector.memset(eps_t[:], EPS)

        sums_t = {}
        sv_t = {}

        def emit_stats_mm(g):
            kp, nsec, nch = GROUP_KP[g], GROUP_NSEC[g], GROUP_NCH[g]
            sums = statsp.tile([128, 2, nch * 2], f32, tag=f"sums{g}",
                               name=f"sums{g}")
            sums_t[g] = sums
            for c in range(nch):
                pssum = psmm.tile([128, 512], f32, tag="mm",
                                  name=f"pssum{g}_{c}")
                for j in range(8):
                    nc.tensor.matmul(
                        pssum[:, 0:NSUB], id_t[:],
                        xtiles[g][c][:, j * NSUB:(j + 1) * NSUB],
                        start=(j == 0), stop=(j == 7),
                        tile_position=(0, 0))
                nc.vector.tensor_reduce(
                    out=sums[:, 0, c:c + 1], in_=pssum[:, 0:NSUB],
                    op=mybir.AluOpType.add, axis=mybir.AxisListType.X)

        def emit_stats_sq(g, c, split):
            sums = sums_t[g]
            for p2 in range(2):
                scr_v = statsp.tile([128, 2000], bf16, tag="scr_v", bufs=3,
                                    name=f"scrv{g}_{c}_{p2}")
                xin = xtiles[g][c][:, p2 * 2000:(p2 + 1) * 2000]
                acc = sums[:, 1, 2 * c + p2:2 * c + p2 + 1]
                if split and p2 == 1:
                    nc.vector.scalar_tensor_tensor(
                        out=scr_v[:], in0=xin, scalar=0.0, in1=xin,
                        op0=mybir.AluOpType.add, op1=mybir.AluOpType.mult,
                        accum_out=acc)
                else:
                    nc.scalar.activation(
                        out=scr_v[:], in_=xin,
                        func=mybir.ActivationFunctionType.Square,
                        bias=0.0, scale=1.0, accum_out=acc)

        def emit_sv_fold(g):
            kp, nsec, nch = GROUP_KP[g], GROUP_NSEC[g], GROUP_NCH[g]
            wg = w_t[:, 128 * g:128 * (g + 1)]
            selg = sel_t[:, 128 * g:128 * (g + 1)]
            biag = gbb_t[:, 3 * g + 2:3 * g + 3]
            sums = sums_t[g]
            sv = vecs.tile([128, 2], f32, tag=f"sv{g}", name=f"sv{g}")
            for j, width in ((0, nch), (1, nch * 2)):
                scr_sv = vecs.tile([128, nch * 2], f32, tag=f"scrsv{g}_{j}",
                                   name=f"scrsv{g}_{j}")
                nc.scalar.activation(
                    out=scr_sv[:, 0:width], in_=sums[:, j, 0:width],
                    func=mybir.ActivationFunctionType.Identity,
                    bias=0.0, scale=1.0, accum_out=sv[:, j:j + 1])
            # combine across sections + broadcast back via selection matmul;
            # result is (mean, E[x^2]) per partition row
            pst = psmm.tile([128, 2], f32, tag="mm", name=f"pst{g}")
            nc.tensor.matmul(pst[:], selg, sv[:], start=True, stop=True)

            # fold BN into conv (gamma/beta are folded on the host: w is
            # W*gamma and bias2 = bias + W^T beta)
            msq2 = vecs.tile([128, 1], f32, tag=f"msq2{g}", name=f"msq2{g}")
            nc.scalar.activation(out=msq2[:], in_=pst[:, 0:1],
                                 func=mybir.ActivationFunctionType.Square,
                                 bias=0.0, scale=1.0)
            var = vecs.tile([128, 1], f32, tag=f"var{g}", name=f"var{g}")
            nc.vector.tensor_sub(out=var[:], in0=pst[:, 1:2], in1=msq2[:])
            std = vecs.tile([128, 1], f32, tag=f"std{g}", name=f"std{g}")
            nc.scalar.activation(out=std[:], in_=var[:],
                                 func=mybir.ActivationFunctionType.Sqrt,
                                 bias=eps_t[:], scale=1.0)
            rstd = vecs.tile([128, 1], f32, tag=f"rstd{g}", name=f"rstd{g}")
            nc.vector.reciprocal(out=rstd[:], in_=std[:])
            wf = wfp.tile([128, 128], bf16, tag=f"wf{g}", name=f"wf{g}")
            nc.vector.tensor_scalar_mul(out=wf[:], in0=wg, scalar1=rstd[:])
            mr = vecs.tile([128, 1], bf16, tag=f"mr{g}", name=f"mr{g}")
            nc.vector.tensor_copy(out=mr[:], in_=pst[:, 0:1])
            psb = psmm.tile([128, 1], f32, tag="mm", name=f"psb{g}")
            nc.tensor.matmul(psb[:], wf[0:kp, :], mr[0:kp, :],
                             start=True, stop=True)
            bf = vecs.tile([128, 1], f32, tag=f"bf{g}", name=f"bf{g}")
            nc.vector.tensor_sub(out=bf[:], in0=biag, in1=psb[:])
            wfs[g] = wf
            bfs[g] = bf

        def emit_main(g, c, qp):
            # matmuls alternate between the two sections of a pair so each
            # LDWEIGHTS hits a different PE row-group than the running matmul
            kp, nsec, nch = GROUP_KP[g], GROUP_NSEC[g], GROUP_NCH[g]
            wf, bf = wfs[g], bfs[g]
            xts = xtiles[g]
            qs = (2 * qp, 2 * qp + 1)
            stage = ostage.tile([128, 2 * T], bf16, tag="stage",
                                name=f"stage{g}_{c}_{qp}")
            for u2 in range(4):
                pss = [psmm.tile([128, 1024], f32, tag="mm",
                                 name=f"ps{g}_{c}_{qp}_{u2}_{qi}")
                       for qi in range(2)]
                for h in range(2):
                    u = u2 * 2 + h
                    for qi, q in enumerate(qs):
                        base = kp * q
                        nc.tensor.matmul(
                            pss[qi][:, 512 * h:512 * h + NSUB],
                            wf[base:base + kp, :],
                            xts[c][base:base + kp,
                                   u * NSUB:(u + 1) * NSUB],
                            start=True, stop=True,
                            tile_position=(base, 0),
                        )
                for qi in range(2):
                    pv = pss[qi][:].rearrange(
                        "p (a b) -> p a b", a=2)[:, :, 0:NSUB]
                    so = stage[:, qi * T + u2 * 1000:
                               qi * T + (u2 + 1) * 1000].rearrange(
                        "p (a b) -> p a b", a=2)
                    if alt[0] % 8 in (0, 2, 4, 6):
                        nc.vector.tensor_scalar_add(out=so, in0=pv,
                                                    scalar1=bf[:])
                    else:
                        nc.scalar.add(out=so, in_=pv, add=bf[:])
                    alt[0] += 1
                if u2 == 1 or u2 == 3:
                    # drain the finished u2-pair (both q halves) early
                    lo = (u2 - 1) * 1000
                    svw = stage[:].rearrange(
                        "p (q n) -> p q n", q=2)[:, :, lo:lo + 2000]
                    dvw = y_d[g][c, qp].rearrange(
                        "p (q n) -> p q n", q=2)[:, :, lo:lo + 2000]
                    nc.gpsimd.dma_start(out=dvw, in_=svw)

        emit_dmas(1)
        emit_consts()
        emit_dmas(0)
        emit_stats_mm(1)
        for c in range(2):
            emit_stats_sq(1, c, split=True)
        emit_sv_fold(1)
        emit_stats_mm(0)
        g1_blocks = [(c, qp) for c in range(GROUP_NCH[1])
                     for qp in range(GROUP_NSEC[1] // 2)]
        for i, (c, qp) in enumerate(g1_blocks):
            emit_main(1, c, qp)
            if i < GROUP_NCH[0]:
                emit_stats_sq(0, i, split=True)
        emit_sv_fold(0)
        for c in range(GROUP_NCH[0]):
            for qp in range(GROUP_NSEC[0] // 2):
                emit_main(0, c, qp)

    nc.compile()
    return nc


def _band_x(spec, i):
    s, sb = BAND_START[i], SUBBANDS[i]
    return spec[:, s:s + sb].reshape(B, 2 * sb, T)


def _make_in_maps(spec, weights, biases, gammas, betas):
    in_maps = []
    for core in range(8):
        im = {}
        w_all = np.zeros((128, 256), np.float32)
        gbb = np.zeros((128, 6), np.float32)
        for g, bands in enumerate(CORE_BANDS[core]):
            kp, nsec, nch = GROUP_KP[g], GROUP_NSEC[g], GROUP_NCH[g]
            xcat = np.concatenate([_band_x(spec, i) for i in bands], axis=1)
            K = xcat.shape[1]
            xgh = np.zeros((nch, 128, T), np.float32)
            for q in range(nsec):
                # section q (partitions kp*q..kp*q+K), chunk c -> batch nch*q+c
                xgh[:, kp * q:kp * q + K, :] = xcat[nch * q:nch * (q + 1)]
            im[f"xg{g}"] = xgh.astype(ml_dtypes.bfloat16)

            blk = np.zeros((kp, 128), np.float32)
            biasv = np.zeros((128,), np.float32)
            off = 0
            for bi, band in enumerate(bands):
                ci = 2 * SUBBANDS[band]
                wb = np.asarray(weights[band], np.float64)
                gb = np.asarray(gammas[band], np.float64)
                bb = np.asarray(betas[band], np.float64)
                blk[off:off + ci, 64 * bi:64 * bi + 64] = (wb * gb).T
                biasv[64 * bi:64 * bi + 64] = (
                    np.asarray(biases[band], np.float64) + wb @ bb
                ).astype(np.float32)
                off += ci
            for q in range(128 // kp):
                w_all[kp * q:kp * (q + 1), 128 * g:128 * (g + 1)] = blk
            gbb[:, 3 * g + 2] = biasv
        im["w"] = w_all.astype(ml_dtypes.bfloat16)
        im["ident"] = np.eye(128, dtype=np.float32).astype(ml_dtypes.bfloat16)
        im["sel"] = np.concatenate(SEL, axis=1)
        im["gbb"] = gbb
        in_maps.append(im)
    return in_maps


def _gather(results):
    out = np.empty((B, 31, C, T), np.float32)
    for core in range(8):
        for g, bands in enumerate(CORE_BANDS[core]):
            nch = GROUP_NCH[g]
            # y_g: [nch, nsec//2, 128, 2, T]; col half j -> batch nch*(2qp+j)+c
            yg = np.asarray(results[core][f"y{g}"]).astype(np.float32)
            yg = yg.reshape(nch, -1, 128, 2, T)
            for bi, band in enumerate(bands):
                rows = yg[:, :, 64 * bi:64 * bi + 64]  # [nch, nsec2, 64, 2, T]
                for c in range(nch):
                    for qp in range(rows.shape[1]):
                        for j in range(2):
                            out[nch * (2 * qp + j) + c, band] = rows[c, qp, :, j]
    return out


def run(trace=False, trace_cores=None, **inputs):
    from concourse.bass_utils import run_bass_kernel_spmd

    spec = np.ascontiguousarray(np.asarray(inputs["spec_noisy"], np.float32))
    in_maps = _make_in_maps(spec, inputs["weights"], inputs["biases"],
                            inputs["gammas"], inputs["betas"])
    if "nc" not in _CACHE:
        _CACHE["nc"] = _build_nc()
    nc = _CACHE["nc"]
    res = run_bass_kernel_spmd(nc, in_maps, core_ids=list(range(8)),
                               trace=trace, trace_cores=trace_cores)
    return _gather(res.results), res


def kernel(**inputs):
    out, _ = run(trace=False, **inputs)
    return out
